# revision 3
# baseline (speedup 1.0000x reference)
"""Trainium2 Bass kernel for scatter-memory GRU update (v2).

reference semantics (single-device jax, CPU):
    current = memory[node_ids]                 # [B, H] gather
    h_new   = GRUCell(messages, current)       # [B, H]
    out     = memory.at[node_ids].set(h_new)   # last occurrence wins

Strategy (8 NeuronCores):
  * Host routes: dedupe node_ids to the last occurrence (jax-CPU
    .at[].set semantics), partition the ~181k unique ids across 8 cores
    by id range (row-wise memory sharding), and host-gathers the
    current memory rows (memory is bf16-cast once).  Host work is the
    sharding/routing layer; all GRU math runs on device.
  * Device per core: stream dense [feature, item] blocks of gathered
    rows hT and routed messages xT, run the GRU fully on-chip, stream
    dense h_new blocks back.  Dense streams replace the v1 SWDGE row
    gather, whose ~100ns/descriptor/engine cost (24.6k row-descriptors
    per core) dominated the old 176us kernel.
  * Host assembles the output: out = memory.copy(); out[u] = h_new
    rows (untouched 82% of rows never move through the device).

GRU dataflow per 512-item chunk (software-pipelined across 45 chunks,
3-chunk double-buffered IO blocks):
    pr = Whh_r.h + Wih_r.x      pz = Whh_z.h + Wih_z.x      (PE)
    pn = Whh_n.h                pg = Wih_n.x  (group open)   (PE)
    r = sigmoid(pr + br)        z = sigmoid(pz + bz)         (ACT)
    t1 = (pn + b_hhn) * r                                    (DVE stt)
    pg += I.t1   (identity matmul closes the group)          (PE)
    n  = tanh(pg + b_ihn)   # same ACT table as sigmoid      (ACT)
    nmh = n - h;  t3 = z * nmh;  out = n - t3                (DVE tt)
"""

import numpy as np

NUM_NODES = 1_000_000
MEM_DIM = 128
N_CORES = 8
ROWS_CORE = NUM_NODES // N_CORES       # 125000
CHUNK = 512                            # items per PSUM bank
K_BLK = 2                              # chunks per IO block
BLK = CHUNK * K_BLK


def _host_prep(node_ids, messages, memory_bf):
    """Dedupe ids (last occurrence wins), route to cores, host-gather
    memory rows. Returns per-core dense [128, capc] hT/xT (block-major)."""
    ids = np.asarray(node_ids).astype(np.int64)
    msgs = np.asarray(messages, dtype=np.float32)
    B = len(ids)
    u, ri = np.unique(ids[::-1], return_index=True)
    win_pos = B - 1 - ri
    bounds = np.searchsorted(u, np.arange(N_CORES + 1) * ROWS_CORE)
    counts = np.diff(bounds)
    capc = int(np.ceil(counts.max() / BLK) * BLK)
    n_blk = capc // BLK

    per_core = []
    for c in range(N_CORES):
        lo, hi = bounds[c], bounds[c + 1]
        n = hi - lo
        hT = np.zeros((MEM_DIM, capc), np.float32)
        xT = np.zeros((MEM_DIM, capc), np.float32)
        hT[:, :n] = memory_bf[u[lo:hi]].astype(np.float32).T
        xT[:, :n] = msgs[win_pos[lo:hi]].T
        # block-major layout: [n_blk, 128, BLK] contiguous per block
        hTb = np.ascontiguousarray(
            hT.reshape(MEM_DIM, n_blk, BLK).transpose(1, 0, 2))
        xTb = np.ascontiguousarray(
            xT.reshape(MEM_DIM, n_blk, BLK).transpose(1, 0, 2))
        per_core.append({"hT": hTb, "xT": xTb})
    meta = {"u": u, "bounds": bounds}
    return per_core, capc, meta





def _make_in_maps(inputs, per_core):
    import ml_dtypes
    bf = ml_dtypes.bfloat16
    W_ih = np.asarray(inputs["W_ih"], dtype=np.float32)
    W_hh = np.asarray(inputs["W_hh"], dtype=np.float32)
    b_ih = np.asarray(inputs["b_ih"], dtype=np.float32)
    b_hh = np.asarray(inputs["b_hh"], dtype=np.float32)

    wT = np.ascontiguousarray(
        np.concatenate([W_ih.T, W_hh.T], axis=1)).astype(bf)   # [128, 768]
    bias = np.stack([
        b_ih[0:128] + b_hh[0:128],
        b_ih[128:256] + b_hh[128:256],
        b_ih[256:384],
        b_hh[256:384],
    ], axis=1).astype(np.float32)                               # [128, 4]
    ident = np.eye(128, dtype=np.float32).astype(bf)

    in_maps = []
    for c in range(N_CORES):
        in_maps.append({
            "hT": per_core[c]["hT"].astype(bf),
            "xT": per_core[c]["xT"].astype(bf),
            "wT": wT,
            "bias": bias,
            "ident": ident,
        })
    return in_maps


def _run(inputs, trace=False):
    import ml_dtypes
    from concourse.bass_utils import run_bass_kernel_spmd
    bf = ml_dtypes.bfloat16

    memory_bf = np.asarray(inputs["memory"], dtype=np.float32).astype(bf)
    per_core, capc, meta = _host_prep(inputs["node_ids"], inputs["messages"],
                                      memory_bf)
    in_maps = _make_in_maps(inputs, per_core)
    nc = _build_program(capc, prefetch_blk=3)
    res = run_bass_kernel_spmd(nc, in_maps, list(range(N_CORES)),
                               trace=trace)

    u, bounds = meta["u"], meta["bounds"]
    outp = np.array(np.asarray(inputs["memory"], dtype=np.float32), copy=True)
    for c in range(N_CORES):
        lo, hi = bounds[c], bounds[c + 1]
        n = hi - lo
        if n:
            hT = np.asarray(res.results[c]["houtT"]).transpose(1, 0, 2)
            hT = hT.reshape(MEM_DIM, capc)
            outp[u[lo:hi]] = hT[:, :n].astype(np.float32).T
    return outp, res


def kernel(**inputs):
    outp, _ = _run(inputs, trace=False)
    return outp


# revision 5
# speedup vs baseline: 1.1557x; 1.1557x over previous
"""Trainium2 Bass kernel for scatter-memory GRU update (v2).

reference semantics (single-device jax, CPU):
    current = memory[node_ids]                 # [B, H] gather
    h_new   = GRUCell(messages, current)       # [B, H]
    out     = memory.at[node_ids].set(h_new)   # last occurrence wins

Strategy (8 NeuronCores):
  * Host routes: dedupe node_ids to the last occurrence (jax-CPU
    .at[].set semantics), partition the ~181k unique ids across 8 cores
    by id range (row-wise memory sharding), and host-gathers the
    current memory rows (memory is bf16-cast once).  Host work is the
    sharding/routing layer; all GRU math runs on device.
  * Device per core: stream dense [feature, item] blocks of gathered
    rows hT and routed messages xT, run the GRU fully on-chip, stream
    dense h_new blocks back.  Dense streams replace the v1 SWDGE row
    gather, whose ~100ns/descriptor/engine cost (24.6k row-descriptors
    per core) dominated the old 176us kernel.
  * Host assembles the output: out = memory.copy(); out[u] = h_new
    rows (untouched 82% of rows never move through the device).

GRU dataflow per 512-item chunk (software-pipelined across 45 chunks,
3-chunk double-buffered IO blocks; engine busy/core: ACT ~84us,
DVE ~74us, PE ~68us, DMA ~50us):
    pr = Whh_r.h + Wih_r.x      pz = Whh_z.h + Wih_z.x      (PE)
    pn = Whh_n.h                pg = Wih_n.x  (group open)   (PE)
    r = sigmoid(pr + br)        z = sigmoid(pz + bz)         (ACT)
    t1 = (pn + b_hhn) * r                                    (DVE stt)
    pg += I.t1   (identity matmul closes the group)          (PE)
    n  = tanh(pg + b_ihn)   # same ACT table set as sigmoid  (ACT)
    nmh = n - h;  t3 = z * nmh;  out = n - t3                (DVE tt)
"""

import numpy as np

NUM_NODES = 1_000_000
MEM_DIM = 128
N_CORES = 8
ROWS_CORE = NUM_NODES // N_CORES       # 125000
CHUNK = 512                            # items per PSUM bank
K_BLK = 3                              # chunks per IO block
BLK = CHUNK * K_BLK
TANH_VIA_SIG = False                   # n-gate uses the Tanh table directly

def _host_prep(node_ids, messages, memory_bf, k_blk=K_BLK):
    """Dedupe ids (last occurrence wins), route to cores, host-gather
    memory rows. Returns per-core dense [128, capc] hT/xT (block-major)."""
    ids = np.asarray(node_ids).astype(np.int64)
    msgs = np.asarray(messages, dtype=np.float32)
    B = len(ids)
    u, ri = np.unique(ids[::-1], return_index=True)
    win_pos = B - 1 - ri
    blk = CHUNK * k_blk
    bounds = np.searchsorted(u, np.arange(N_CORES + 1) * ROWS_CORE)
    counts = np.diff(bounds)
    capc = int(np.ceil(counts.max() / blk) * blk)
    n_blk = capc // blk

    per_core = []
    for c in range(N_CORES):
        lo, hi = bounds[c], bounds[c + 1]
        n = hi - lo
        hT = np.zeros((MEM_DIM, capc), np.float32)
        xT = np.zeros((MEM_DIM, capc), np.float32)
        hT[:, :n] = memory_bf[u[lo:hi]].astype(np.float32).T
        xT[:, :n] = msgs[win_pos[lo:hi]].T
        # block-major layout: [n_blk, 128, BLK] contiguous per block
        hTb = np.ascontiguousarray(
            hT.reshape(MEM_DIM, n_blk, blk).transpose(1, 0, 2))
        xTb = np.ascontiguousarray(
            xT.reshape(MEM_DIM, n_blk, blk).transpose(1, 0, 2))
        per_core.append({"hT": hTb, "xT": xTb})
    meta = {"u": u, "bounds": bounds}
    return per_core, capc, meta


def _build_program(capc, repeats=1, t1_engine="dve", prefetch_blk=2,
                   ablate=(), loop_mode="unroll", tanh_sig=None,
                   a_engine="dve", t3_engine="dve", fin_engine="dve",
                   lag_t2=1, lag_mid2=1, lag_final=2, wbufs=4,
                   psum=(2, 2, 1, 3), k_blk=K_BLK):
    import concourse.bass as bass
    import concourse.bacc as bacc
    import concourse.mybir as mybir
    import concourse.tile as tile

    f32 = mybir.dt.float32
    bf16 = mybir.dt.bfloat16
    AF = mybir.ActivationFunctionType
    ALU = mybir.AluOpType
    n_chunks = capc // CHUNK
    K_B = k_blk
    BLKL = CHUNK * k_blk
    n_blk = capc // BLKL
    C = CHUNK

    nc = bacc.Bacc(None, target_bir_lowering=False)
    hT_d = nc.declare_dram_parameter("hT", [n_blk, MEM_DIM, BLKL], bf16,
                                     isOutput=False)
    xT_d = nc.declare_dram_parameter("xT", [n_blk, MEM_DIM, BLKL], bf16,
                                     isOutput=False)
    wT_d = nc.declare_dram_parameter("wT", [MEM_DIM, 6 * MEM_DIM], bf16,
                                     isOutput=False)
    bias_d = nc.declare_dram_parameter("bias", [MEM_DIM, 5], f32,
                                       isOutput=False)
    ident_d = nc.declare_dram_parameter("ident", [128, 128], bf16,
                                        isOutput=False)
    outT_d = nc.declare_dram_parameter("houtT", [n_blk, MEM_DIM, BLKL], bf16,
                                       isOutput=True)

    if tanh_sig is None:
        tanh_sig = TANH_VIA_SIG
    with tile.TileContext(nc) as tc:
        with (
            tc.tile_pool(name="const", bufs=1) as cpool,
            tc.tile_pool(name="h", bufs=prefetch_blk + 1) as hpool,
            tc.tile_pool(name="msg", bufs=prefetch_blk + 1) as mpool,
            tc.tile_pool(name="o", bufs=2) as opool,
            tc.tile_pool(name="work", bufs=wbufs) as wpool,
            tc.tile_pool(name="psR", bufs=psum[0], space="PSUM") as ppoolR,
            tc.tile_pool(name="psN", bufs=psum[1], space="PSUM") as ppoolN,
            tc.tile_pool(name="psZ", bufs=psum[2], space="PSUM") as ppoolZ,
            tc.tile_pool(name="psG", bufs=psum[3], space="PSUM") as ppoolG,
        ):
            w_sb = cpool.tile([128, 6 * MEM_DIM], bf16)
            nc.sync.dma_start(out=w_sb[:], in_=wT_d[:])
            b_sb = cpool.tile([128, 5], f32)
            nc.sync.dma_start(out=b_sb[:], in_=bias_d[:])
            ident = cpool.tile([128, 128], bf16)
            nc.sync.dma_start(out=ident[:], in_=ident_d[:])
            dummy = None
            if "act" in ablate or "dve" in ablate:
                dummy = cpool.tile([128, C], bf16)
                nc.gpsimd.memset(dummy[:], 0.25)

            engs = {"dve": nc.vector, "pool": nc.gpsimd}
            t1eng = engs[t1_engine]
            a_eng = engs[a_engine]
            t3_eng = engs[t3_engine]
            fin_eng = engs[fin_engine]

            def emit_front(g, st):
                s = st[g]
                hc, xc = s["hc"], s["xc"]
                pr = ppoolR.tile([128, C], f32, tag="pr")
                nc.tensor.matmul(pr[:], lhsT=w_sb[:, 384:512], rhs=hc,
                                 start=True, stop=False)
                nc.tensor.matmul(pr[:], lhsT=w_sb[:, 0:128], rhs=xc,
                                 start=False, stop=True)
                pz = ppoolZ.tile([128, C], f32, tag="pz")
                nc.tensor.matmul(pz[:], lhsT=w_sb[:, 512:640], rhs=hc,
                                 start=True, stop=False)
                nc.tensor.matmul(pz[:], lhsT=w_sb[:, 128:256], rhs=xc,
                                 start=False, stop=True)
                pn = ppoolN.tile([128, C], f32, tag="pn")
                nc.tensor.matmul(pn[:], lhsT=w_sb[:, 640:768], rhs=hc,
                                 start=True, stop=True)
                pg = ppoolG.tile([128, C], f32, tag="pg")
                nc.tensor.matmul(pg[:], lhsT=w_sb[:, 256:384], rhs=xc,
                                 start=True, stop=False)
                if "act" in ablate:
                    r = z = dummy
                else:
                    r = wpool.tile([128, C], bf16, tag="r")
                    nc.scalar.activation(r[:], pr[:], AF.Sigmoid,
                                         bias=b_sb[:, 0:1])
                    z = wpool.tile([128, C], bf16, tag="z")
                    nc.scalar.activation(z[:], pz[:], AF.Sigmoid,
                                         bias=b_sb[:, 1:2])
                if "dve" in ablate:
                    t1 = dummy
                else:
                    t1 = wpool.tile([128, C], bf16, tag="t1")
                    t1eng.scalar_tensor_tensor(t1[:], pn[:], b_sb[:, 3:4],
                                               r[:], op0=ALU.add,
                                               op1=ALU.mult)
                s.update(pg=pg, t1=t1, z=z)

            def emit_t2(g, st):
                s = st[g]
                nc.tensor.matmul(s["pg"][:], lhsT=ident[:], rhs=s["t1"][:],
                                 start=False, stop=True)

            def emit_mid2(g, st):
                s = st[g]
                hc = s["hc"]
                if "act" in ablate:
                    n = dummy
                elif tanh_sig:
                    n = wpool.tile([128, C], bf16, tag="n")
                    nc.scalar.activation(n[:], s["pg"][:], AF.Sigmoid,
                                         bias=b_sb[:, 4:5], scale=2.0)
                else:
                    n = wpool.tile([128, C], bf16, tag="n")
                    nc.scalar.activation(n[:], s["pg"][:], AF.Tanh,
                                         bias=b_sb[:, 2:3])
                if "dve" in ablate:
                    t3 = dummy
                elif tanh_sig:
                    t3 = wpool.tile([128, C], bf16, tag="t3")
                    a = wpool.tile([128, C], bf16, tag="nmh")
                    a_eng.scalar_tensor_tensor(
                        a[:], n[:], 2.0, hc, op0=ALU.mult,
                        op1=ALU.subtract)          # 2s - h
                    t3_eng.scalar_tensor_tensor(
                        t3[:], a[:], -1.0, s["z"][:], op0=ALU.add,
                        op1=ALU.mult)              # (2s-h-1)*z = z*(n-h)
                else:
                    t3 = wpool.tile([128, C], bf16, tag="t3")
                    a = wpool.tile([128, C], bf16, tag="nmh")
                    a_eng.tensor_sub(a[:], n[:], hc)       # n - h
                    t3_eng.tensor_mul(t3[:], s["z"][:], a[:])  # z*(n-h)
                s.update(n=n, t3=t3)

            def emit_final(g, st):
                s = st.pop(g)
                b, c = divmod(g, K_B)
                i0 = c * C
                if "dve" not in ablate:
                    if tanh_sig:
                        fin_eng.scalar_tensor_tensor(
                            s["oT"][:, i0:i0 + C], s["n"][:], 2.0,
                            s["t3"][:], op0=ALU.mult, op1=ALU.subtract)
                    else:
                        fin_eng.tensor_sub(s["oT"][:, i0:i0 + C],
                                           s["n"][:], s["t3"][:])
                if c == K_B - 1 and ("dve" not in ablate):
                    nc.sync.dma_start(out=outT_d[b], in_=s["oT"][:])

            def emit_body():
                hts, mts, ots = {}, {}, {}

                def emit_load(b):
                    hts[b] = hpool.tile([128, BLKL], bf16, tag="hT",
                                        name=f"hT{b}")
                    mts[b] = mpool.tile([128, BLKL], bf16, tag="xT",
                                        name=f"xT{b}")
                    nc.sync.dma_start(out=hts[b][:], in_=hT_d[b])
                    nc.sync.dma_start(out=mts[b][:], in_=xT_d[b])

                for b in range(min(prefetch_blk, n_blk)):
                    emit_load(b)
                st = {}
                for g in range(n_chunks):
                    b, c = divmod(g, K_B)
                    if c == 0:
                        if b + prefetch_blk < n_blk:
                            emit_load(b + prefetch_blk)
                        if "compute" not in ablate and "dve" not in ablate:
                            ots[b] = opool.tile([128, BLKL], bf16, tag="oT",
                                                name=f"oT{b}")
                    if "compute" in ablate:
                        continue
                    i0 = c * C
                    st[g] = {"hc": hts[b][:, i0:i0 + C],
                             "xc": mts[b][:, i0:i0 + C],
                             "oT": ots.get(b)}
                    if g >= lag_t2:
                        emit_t2(g - lag_t2, st)
                    if g >= lag_final:
                        emit_final(g - lag_final, st)
                    emit_front(g, st)
                    if g >= lag_mid2:
                        emit_mid2(g - lag_mid2, st)
                if "compute" in ablate:
                    return
                for g in range(n_chunks - lag_t2, n_chunks):
                    emit_t2(g, st)
                for g in range(n_chunks - lag_mid2, n_chunks):
                    emit_mid2(g, st)
                for g in range(n_chunks - lag_final, n_chunks):
                    emit_final(g, st)

            if repeats == 1:
                emit_body()
            elif loop_mode == "for_i":
                with tc.For_i(0, repeats):
                    emit_body()
            else:
                for rep in range(repeats):
                    if rep:
                        tc.strict_bb_all_engine_barrier()
                    emit_body()
    nc.compile()
    return nc


def _make_in_maps(inputs, per_core):
    import ml_dtypes
    bf = ml_dtypes.bfloat16
    W_ih = np.asarray(inputs["W_ih"], dtype=np.float32)
    W_hh = np.asarray(inputs["W_hh"], dtype=np.float32)
    b_ih = np.asarray(inputs["b_ih"], dtype=np.float32)
    b_hh = np.asarray(inputs["b_hh"], dtype=np.float32)

    wT = np.ascontiguousarray(
        np.concatenate([W_ih.T, W_hh.T], axis=1)).astype(bf)   # [128, 768]
    bias = np.stack([
        b_ih[0:128] + b_hh[0:128],
        b_ih[128:256] + b_hh[128:256],
        b_ih[256:384],
        b_hh[256:384],
        2.0 * b_ih[256:384],
    ], axis=1).astype(np.float32)                               # [128, 5]
    ident = np.eye(128, dtype=np.float32).astype(bf)

    in_maps = []
    for c in range(N_CORES):
        in_maps.append({
            "hT": per_core[c]["hT"].astype(bf),
            "xT": per_core[c]["xT"].astype(bf),
            "wT": wT,
            "bias": bias,
            "ident": ident,
        })
    return in_maps


def _run(inputs, trace=False):
    import ml_dtypes
    from concourse.bass_utils import run_bass_kernel_spmd
    bf = ml_dtypes.bfloat16

    memory_bf = np.asarray(inputs["memory"], dtype=np.float32).astype(bf)
    per_core, capc, meta = _host_prep(inputs["node_ids"], inputs["messages"],
                                      memory_bf)
    in_maps = _make_in_maps(inputs, per_core)
    nc = _build_program(capc, tanh_sig=False)
    res = run_bass_kernel_spmd(nc, in_maps, list(range(N_CORES)),
                               trace=trace)

    u, bounds = meta["u"], meta["bounds"]
    outp = np.array(np.asarray(inputs["memory"], dtype=np.float32), copy=True)
    n_blk = capc // BLK
    for c in range(N_CORES):
        lo, hi = bounds[c], bounds[c + 1]
        n = hi - lo
        if n:
            hT = np.asarray(res.results[c]["houtT"]).transpose(1, 0, 2)
            hT = hT.reshape(MEM_DIM, capc)
            outp[u[lo:hi]] = hT[:, :n].astype(np.float32).T
    return outp, res


def kernel(**inputs):
    outp, _ = _run(inputs, trace=False)
    return outp

# revision 6
# speedup vs baseline: 1.1916x; 1.0311x over previous
"""Trainium2 Bass kernel for scatter-memory GRU update (v2).

reference semantics (single-device jax, CPU):
    current = memory[node_ids]                 # [B, H] gather
    h_new   = GRUCell(messages, current)       # [B, H]
    out     = memory.at[node_ids].set(h_new)   # last occurrence wins

Strategy (8 NeuronCores):
  * Host routes: dedupe node_ids to the last occurrence (jax-CPU
    .at[].set semantics), partition the ~181k unique ids across 8 cores
    by id range (row-wise memory sharding), and host-gathers the
    current memory rows (memory is bf16-cast once).  Host work is the
    sharding/routing layer; all GRU math runs on device.
  * Device per core: stream dense [feature, item] blocks of gathered
    rows hT and routed messages xT, run the GRU fully on-chip, stream
    dense h_new blocks back.  Dense streams replace the v1 SWDGE row
    gather, whose ~100ns/descriptor/engine cost (24.6k row-descriptors
    per core) dominated the old 176us kernel.
  * Host assembles the output: out = memory.copy(); out[u] = h_new
    rows (untouched 82% of rows never move through the device).

GRU dataflow per 512-item chunk (software-pipelined across 45 chunks,
3-chunk double-buffered IO blocks; engine busy/core: ACT ~84us,
DVE ~74us, PE ~68us, DMA ~50us):
    pr = Whh_r.h + Wih_r.x      pz = Whh_z.h + Wih_z.x      (PE)
    pn = Whh_n.h                pg = Wih_n.x  (group open)   (PE)
    r = sigmoid(pr + br)        z = sigmoid(pz + bz)         (ACT)
    t1 = (pn + b_hhn) * r                                    (DVE stt)
    pg += I.t1   (identity matmul closes the group)          (PE)
    n  = tanh(pg + b_ihn)   # same ACT table set as sigmoid  (ACT)
    nmh = n - h;  t3 = z * nmh;  out = n - t3                (DVE tt)
"""

import numpy as np

NUM_NODES = 1_000_000
MEM_DIM = 128
N_CORES = 8
ROWS_CORE = NUM_NODES // N_CORES       # 125000
CHUNK = 512                            # items per PSUM bank
K_BLK = 3                              # chunks per IO block
BLK = CHUNK * K_BLK
TANH_VIA_SIG = False                   # n-gate uses the Tanh table directly

def _host_prep(node_ids, messages, memory_bf, k_blk=K_BLK):
    """Dedupe ids (last occurrence wins), route to cores, host-gather
    memory rows. Returns per-core dense [128, capc] hT/xT (block-major)."""
    ids = np.asarray(node_ids).astype(np.int64)
    msgs = np.asarray(messages, dtype=np.float32)
    B = len(ids)
    u, ri = np.unique(ids[::-1], return_index=True)
    win_pos = B - 1 - ri
    blk = CHUNK * k_blk
    bounds = np.searchsorted(u, np.arange(N_CORES + 1) * ROWS_CORE)
    counts = np.diff(bounds)
    capc = int(np.ceil(counts.max() / blk) * blk)
    n_blk = capc // blk

    per_core = []
    for c in range(N_CORES):
        lo, hi = bounds[c], bounds[c + 1]
        n = hi - lo
        hT = np.zeros((MEM_DIM, capc), np.float32)
        xT = np.zeros((MEM_DIM, capc), np.float32)
        hT[:, :n] = memory_bf[u[lo:hi]].astype(np.float32).T
        xT[:, :n] = msgs[win_pos[lo:hi]].T
        # block-major layout: [n_blk, 128, BLK] contiguous per block
        hTb = np.ascontiguousarray(
            hT.reshape(MEM_DIM, n_blk, blk).transpose(1, 0, 2))
        xTb = np.ascontiguousarray(
            xT.reshape(MEM_DIM, n_blk, blk).transpose(1, 0, 2))
        per_core.append({"hT": hTb, "xT": xTb})
    meta = {"u": u, "bounds": bounds}
    return per_core, capc, meta


def _build_program(capc, repeats=1, t1_engine="dve", prefetch_blk=2,
                   ablate=(), loop_mode="unroll", tanh_sig=None,
                   a_engine="dve", t3_engine="dve", fin_engine="dve",
                   lag_t2=1, lag_mid2=1, lag_final=2, wbufs=4,
                   psum=(2, 2, 2, 2), k_blk=K_BLK):
    import concourse.bass as bass
    import concourse.bacc as bacc
    import concourse.mybir as mybir
    import concourse.tile as tile

    f32 = mybir.dt.float32
    bf16 = mybir.dt.bfloat16
    AF = mybir.ActivationFunctionType
    ALU = mybir.AluOpType
    n_chunks = capc // CHUNK
    K_B = k_blk
    BLKL = CHUNK * k_blk
    n_blk = capc // BLKL
    C = CHUNK

    nc = bacc.Bacc(None, target_bir_lowering=False)
    hT_d = nc.declare_dram_parameter("hT", [n_blk, MEM_DIM, BLKL], bf16,
                                     isOutput=False)
    xT_d = nc.declare_dram_parameter("xT", [n_blk, MEM_DIM, BLKL], bf16,
                                     isOutput=False)
    wT_d = nc.declare_dram_parameter("wT", [MEM_DIM, 6 * MEM_DIM], bf16,
                                     isOutput=False)
    bias_d = nc.declare_dram_parameter("bias", [MEM_DIM, 5], f32,
                                       isOutput=False)
    ident_d = nc.declare_dram_parameter("ident", [128, 128], bf16,
                                        isOutput=False)
    outT_d = nc.declare_dram_parameter("houtT", [n_blk, MEM_DIM, BLKL], bf16,
                                       isOutput=True)

    if tanh_sig is None:
        tanh_sig = TANH_VIA_SIG
    with tile.TileContext(nc) as tc:
        with (
            tc.tile_pool(name="const", bufs=1) as cpool,
            tc.tile_pool(name="h", bufs=prefetch_blk + 1) as hpool,
            tc.tile_pool(name="msg", bufs=prefetch_blk + 1) as mpool,
            tc.tile_pool(name="o", bufs=2) as opool,
            tc.tile_pool(name="work", bufs=wbufs) as wpool,
            tc.tile_pool(name="psR", bufs=psum[0], space="PSUM") as ppoolR,
            tc.tile_pool(name="psN", bufs=psum[1], space="PSUM") as ppoolN,
            tc.tile_pool(name="psZ", bufs=psum[2], space="PSUM") as ppoolZ,
            tc.tile_pool(name="psG", bufs=psum[3], space="PSUM") as ppoolG,
        ):
            w_sb = cpool.tile([128, 6 * MEM_DIM], bf16)
            nc.sync.dma_start(out=w_sb[:], in_=wT_d[:])
            b_sb = cpool.tile([128, 5], f32)
            nc.sync.dma_start(out=b_sb[:], in_=bias_d[:])
            ident = cpool.tile([128, 128], bf16)
            nc.sync.dma_start(out=ident[:], in_=ident_d[:])
            dummy = None
            if "act" in ablate or "dve" in ablate:
                dummy = cpool.tile([128, C], bf16)
                nc.gpsimd.memset(dummy[:], 0.25)

            engs = {"dve": nc.vector, "pool": nc.gpsimd}
            t1eng = engs[t1_engine]
            a_eng = engs[a_engine]
            t3_eng = engs[t3_engine]
            fin_eng = engs[fin_engine]

            def emit_front(g, st):
                s = st[g]
                hc, xc = s["hc"], s["xc"]
                pr = ppoolR.tile([128, C], f32, tag="pr")
                nc.tensor.matmul(pr[:], lhsT=w_sb[:, 384:512], rhs=hc,
                                 start=True, stop=False)
                nc.tensor.matmul(pr[:], lhsT=w_sb[:, 0:128], rhs=xc,
                                 start=False, stop=True)
                pz = ppoolZ.tile([128, C], f32, tag="pz")
                nc.tensor.matmul(pz[:], lhsT=w_sb[:, 512:640], rhs=hc,
                                 start=True, stop=False)
                nc.tensor.matmul(pz[:], lhsT=w_sb[:, 128:256], rhs=xc,
                                 start=False, stop=True)
                pn = ppoolN.tile([128, C], f32, tag="pn")
                nc.tensor.matmul(pn[:], lhsT=w_sb[:, 640:768], rhs=hc,
                                 start=True, stop=True)
                pg = ppoolG.tile([128, C], f32, tag="pg")
                nc.tensor.matmul(pg[:], lhsT=w_sb[:, 256:384], rhs=xc,
                                 start=True, stop=False)
                if "act" in ablate:
                    r = z = dummy
                else:
                    r = wpool.tile([128, C], bf16, tag="r")
                    nc.scalar.activation(r[:], pr[:], AF.Sigmoid,
                                         bias=b_sb[:, 0:1])
                    z = wpool.tile([128, C], bf16, tag="z")
                    nc.scalar.activation(z[:], pz[:], AF.Sigmoid,
                                         bias=b_sb[:, 1:2])
                if "dve" in ablate:
                    t1 = dummy
                else:
                    t1 = wpool.tile([128, C], bf16, tag="t1")
                    t1eng.scalar_tensor_tensor(t1[:], pn[:], b_sb[:, 3:4],
                                               r[:], op0=ALU.add,
                                               op1=ALU.mult)
                s.update(pg=pg, t1=t1, z=z)

            def emit_t2(g, st):
                s = st[g]
                nc.tensor.matmul(s["pg"][:], lhsT=ident[:], rhs=s["t1"][:],
                                 start=False, stop=True)

            def emit_mid2(g, st):
                s = st[g]
                hc = s["hc"]
                if "act" in ablate:
                    n = dummy
                elif tanh_sig:
                    n = wpool.tile([128, C], bf16, tag="n")
                    nc.scalar.activation(n[:], s["pg"][:], AF.Sigmoid,
                                         bias=b_sb[:, 4:5], scale=2.0)
                else:
                    n = wpool.tile([128, C], bf16, tag="n")
                    nc.scalar.activation(n[:], s["pg"][:], AF.Tanh,
                                         bias=b_sb[:, 2:3])
                if "dve" in ablate:
                    t3 = dummy
                elif tanh_sig:
                    t3 = wpool.tile([128, C], bf16, tag="t3")
                    a = wpool.tile([128, C], bf16, tag="nmh")
                    a_eng.scalar_tensor_tensor(
                        a[:], n[:], 2.0, hc, op0=ALU.mult,
                        op1=ALU.subtract)          # 2s - h
                    t3_eng.scalar_tensor_tensor(
                        t3[:], a[:], -1.0, s["z"][:], op0=ALU.add,
                        op1=ALU.mult)              # (2s-h-1)*z = z*(n-h)
                else:
                    t3 = wpool.tile([128, C], bf16, tag="t3")
                    a = wpool.tile([128, C], bf16, tag="nmh")
                    a_eng.tensor_sub(a[:], n[:], hc)       # n - h
                    t3_eng.tensor_mul(t3[:], s["z"][:], a[:])  # z*(n-h)
                s.update(n=n, t3=t3)

            def emit_final(g, st):
                s = st.pop(g)
                b, c = divmod(g, K_B)
                i0 = c * C
                if "dve" not in ablate:
                    if tanh_sig:
                        fin_eng.scalar_tensor_tensor(
                            s["oT"][:, i0:i0 + C], s["n"][:], 2.0,
                            s["t3"][:], op0=ALU.mult, op1=ALU.subtract)
                    else:
                        fin_eng.tensor_sub(s["oT"][:, i0:i0 + C],
                                           s["n"][:], s["t3"][:])
                if c == K_B - 1 and ("dve" not in ablate):
                    nc.sync.dma_start(out=outT_d[b], in_=s["oT"][:])

            def emit_body():
                hts, mts, ots = {}, {}, {}

                def emit_load(b):
                    hts[b] = hpool.tile([128, BLKL], bf16, tag="hT",
                                        name=f"hT{b}")
                    mts[b] = mpool.tile([128, BLKL], bf16, tag="xT",
                                        name=f"xT{b}")
                    nc.sync.dma_start(out=hts[b][:], in_=hT_d[b])
                    nc.sync.dma_start(out=mts[b][:], in_=xT_d[b])

                for b in range(min(prefetch_blk, n_blk)):
                    emit_load(b)
                st = {}
                for g in range(n_chunks):
                    b, c = divmod(g, K_B)
                    if c == 0:
                        if b + prefetch_blk < n_blk:
                            emit_load(b + prefetch_blk)
                        if "compute" not in ablate and "dve" not in ablate:
                            ots[b] = opool.tile([128, BLKL], bf16, tag="oT",
                                                name=f"oT{b}")
                    if "compute" in ablate:
                        continue
                    i0 = c * C
                    st[g] = {"hc": hts[b][:, i0:i0 + C],
                             "xc": mts[b][:, i0:i0 + C],
                             "oT": ots.get(b)}
                    if g >= lag_t2:
                        emit_t2(g - lag_t2, st)
                    if g >= lag_final:
                        emit_final(g - lag_final, st)
                    emit_front(g, st)
                    if g >= lag_mid2:
                        emit_mid2(g - lag_mid2, st)
                if "compute" in ablate:
                    return
                for g in range(n_chunks - lag_t2, n_chunks):
                    emit_t2(g, st)
                for g in range(n_chunks - lag_mid2, n_chunks):
                    emit_mid2(g, st)
                for g in range(n_chunks - lag_final, n_chunks):
                    emit_final(g, st)

            if repeats == 1:
                emit_body()
            elif loop_mode == "for_i":
                with tc.For_i(0, repeats):
                    emit_body()
            else:
                for rep in range(repeats):
                    if rep:
                        tc.strict_bb_all_engine_barrier()
                    emit_body()
    nc.compile()
    return nc


def _make_in_maps(inputs, per_core):
    import ml_dtypes
    bf = ml_dtypes.bfloat16
    W_ih = np.asarray(inputs["W_ih"], dtype=np.float32)
    W_hh = np.asarray(inputs["W_hh"], dtype=np.float32)
    b_ih = np.asarray(inputs["b_ih"], dtype=np.float32)
    b_hh = np.asarray(inputs["b_hh"], dtype=np.float32)

    wT = np.ascontiguousarray(
        np.concatenate([W_ih.T, W_hh.T], axis=1)).astype(bf)   # [128, 768]
    bias = np.stack([
        b_ih[0:128] + b_hh[0:128],
        b_ih[128:256] + b_hh[128:256],
        b_ih[256:384],
        b_hh[256:384],
        2.0 * b_ih[256:384],
    ], axis=1).astype(np.float32)                               # [128, 5]
    ident = np.eye(128, dtype=np.float32).astype(bf)

    in_maps = []
    for c in range(N_CORES):
        in_maps.append({
            "hT": per_core[c]["hT"].astype(bf),
            "xT": per_core[c]["xT"].astype(bf),
            "wT": wT,
            "bias": bias,
            "ident": ident,
        })
    return in_maps


def _run(inputs, trace=False):
    import ml_dtypes
    from concourse.bass_utils import run_bass_kernel_spmd
    bf = ml_dtypes.bfloat16

    memory_bf = np.asarray(inputs["memory"], dtype=np.float32).astype(bf)
    per_core, capc, meta = _host_prep(inputs["node_ids"], inputs["messages"],
                                      memory_bf)
    in_maps = _make_in_maps(inputs, per_core)
    nc = _build_program(capc, tanh_sig=False)
    res = run_bass_kernel_spmd(nc, in_maps, list(range(N_CORES)),
                               trace=trace)

    u, bounds = meta["u"], meta["bounds"]
    outp = np.array(np.asarray(inputs["memory"], dtype=np.float32), copy=True)
    n_blk = capc // BLK
    for c in range(N_CORES):
        lo, hi = bounds[c], bounds[c + 1]
        n = hi - lo
        if n:
            hT = np.asarray(res.results[c]["houtT"]).transpose(1, 0, 2)
            hT = hT.reshape(MEM_DIM, capc)
            outp[u[lo:hi]] = hT[:, :n].astype(np.float32).T
    return outp, res


def kernel(**inputs):
    outp, _ = _run(inputs, trace=False)
    return outp